# revision 1
# baseline (speedup 1.0000x reference)
"""GAU block kernel for 8 trn2 cores.

Sharding: core c = 2*b + h -> batch b (4), tensor-parallel half h (2) of E.
Each core computes partial[b,h] = (u_h * (relu(q kT / L)^2 @ v_h)) @ W_out_h
for its batch; host sums halves + residual + b_out. No collectives.

Layouts (per core, all seq L=2048, D=1024, E_half=1024, S=128):
  xnT  [D_part, L]   normalized-x transposed (PE transpose, rs fused on host.. no: DVE)
  u    spilled to DRAM transposed [E_half, L]
  v    token-major   [L_part, E_half]
  z/q/k transposed   [S, L]
  scores computed as attn^T so every matmul chains without transposes.
All matmul operands float32r (1 cycle/row at N=512); everything else fp32.
"""

import numpy as np
import concourse.bass as bass
import concourse.bacc as bacc
import concourse.mybir as mybir
from contextlib import ExitStack
from concourse.tile import TileContext
from concourse.masks import make_identity
from concourse.bass_utils import run_bass_kernel_spmd

P = 128
L = 2048          # seq len
D = 1024          # model dim
E = 2048          # expansion
EH = E // 2       # per-core half of E
S = 128
KC = D // P       # 8 contraction chunks
ECN = EH // P     # 8 e-chunks
LCN = L // P      # 16 token chunks
G = 4             # l1 groups
GW = L // G       # 512
EPS = 1e-5
f32 = mybir.dt.float32
f32r = mybir.dt.float32r
AF = mybir.ActivationFunctionType
X_AX = mybir.AxisListType.X

LAST_EXEC_NS = None
LAST_WALL_S = None
_CACHE = {}


def _build(has_bv: bool):
    nc = bacc.Bacc(None, target_bir_lowering=False)
    x = nc.declare_dram_parameter("x", [L, D], f32, isOutput=False)
    wu = nc.declare_dram_parameter("wu", [D, EH], f32r, isOutput=False)
    wv = nc.declare_dram_parameter("wv", [D, EH], f32r, isOutput=False)
    wz = nc.declare_dram_parameter("wz", [D, S], f32r, isOutput=False)
    wo = nc.declare_dram_parameter("wo", [EH, D], f32r, isOutput=False)
    bu = nc.declare_dram_parameter("bu", [EH], f32, isOutput=False)
    bz = nc.declare_dram_parameter("bz", [S], f32, isOutput=False)
    gq = nc.declare_dram_parameter("gq", [S], f32, isOutput=False)
    bq = nc.declare_dram_parameter("bq", [S], f32, isOutput=False)
    gk = nc.declare_dram_parameter("gk", [S], f32, isOutput=False)
    bk = nc.declare_dram_parameter("bk", [S], f32, isOutput=False)
    if has_bv:
        bv = nc.declare_dram_parameter("bv", [EH], f32r, isOutput=False)
    part = nc.declare_dram_parameter("part", [L, D], f32, isOutput=True)
    uTd = nc.dram_tensor("uTd", [EH, L], f32r)

    with TileContext(nc) as tc, ExitStack() as top:
        pers = top.enter_context(tc.tile_pool(name="pers", bufs=1))
        v_sb = pers.tile([P, LCN, EH], f32r, name="v_sb")
        qT = pers.tile([P, L], f32r, name="qT")
        kT = pers.tile([P, L], f32r, name="kT")
        ident = pers.tile([P, P], f32, name="ident")
        make_identity(nc, ident[:])
        zero_t = pers.tile([P, 1], f32, name="zero_t")
        nc.vector.memset(zero_t[:], 0.0)
        eps_t = pers.tile([P, 1], f32, name="eps_t")
        nc.vector.memset(eps_t[:], EPS)
        bu_sb = pers.tile([P, ECN], f32, name="bu_sb")
        nc.sync.dma_start(bu_sb[:], bu.rearrange("(ec p) -> p ec", p=P))
        bz_sb = pers.tile([P, 1], f32, name="bz_sb")
        nc.sync.dma_start(bz_sb[:], bz.rearrange("(p o) -> p o", o=1))
        gq_sb = pers.tile([P, 1], f32, name="gq_sb")
        nc.sync.dma_start(gq_sb[:], gq.rearrange("(p o) -> p o", o=1))
        bq_sb = pers.tile([P, 1], f32, name="bq_sb")
        nc.sync.dma_start(bq_sb[:], bq.rearrange("(p o) -> p o", o=1))
        gk_sb = pers.tile([P, 1], f32, name="gk_sb")
        nc.sync.dma_start(gk_sb[:], gk.rearrange("(p o) -> p o", o=1))
        bk_sb = pers.tile([P, 1], f32, name="bk_sb")
        nc.sync.dma_start(bk_sb[:], bk.rearrange("(p o) -> p o", o=1))
        if has_bv:
            ones_t = pers.tile([1, P], f32r, name="ones_t")
            nc.vector.memset(ones_t[:], 1.0)
            bv_sb = pers.tile([1, EH], f32r, name="bv_sb")
            nc.sync.dma_start(bv_sb[:], bv.rearrange("(o e) -> o e", o=1))

        # ---------------- phase 1+2: LN + transpose + projections -------
        with ExitStack() as ctx2:
            xnp = ctx2.enter_context(tc.tile_pool(name="xnp", bufs=1))
            xnT = xnp.tile([P, KC, L], f32r, name="xnT")
            ztp = ctx2.enter_context(tc.tile_pool(name="ztp", bufs=2))
            lnp = ctx2.enter_context(tc.tile_pool(name="lnp", bufs=2))
            wup = ctx2.enter_context(tc.tile_pool(name="wup", bufs=2))
            wvp = ctx2.enter_context(tc.tile_pool(name="wvp", bufs=1))
            wzp = ctx2.enter_context(tc.tile_pool(name="wzp", bufs=1))
            utp = ctx2.enter_context(tc.tile_pool(name="utp", bufs=3))
            pp_ln = ctx2.enter_context(tc.tile_pool(name="pp_ln", bufs=2, space="PSUM"))
            pp_z = ctx2.enter_context(tc.tile_pool(name="pp_z", bufs=1, space="PSUM"))
            pp_u = ctx2.enter_context(tc.tile_pool(name="pp_u", bufs=2, space="PSUM"))
            pp_v = ctx2.enter_context(tc.tile_pool(name="pp_v", bufs=2, space="PSUM"))

            wz_t = wzp.tile([P, KC, S], f32r, name="wz_t")
            nc.sync.dma_start(wz_t[:], wz.rearrange("(kc p) s -> p kc s", p=P))

            for t in range(LCN):
                x_t = lnp.tile([P, D], f32, name="x_t")
                nc.sync.dma_start(x_t[:], x[t * P:(t + 1) * P, :])
                nm = lnp.tile([P, 1], f32, name="nm")
                nc.vector.reduce_sum(nm[:], x_t[:], axis=X_AX)
                nc.scalar.mul(nm[:], nm[:], -1.0 / D)
                xc = lnp.tile([P, D], f32, name="xc")
                nc.vector.tensor_scalar_add(xc[:], x_t[:], nm[:])
                nc.scalar.activation(x_t[:], xc[:], AF.Square, bias=zero_t[:])
                vs = lnp.tile([P, 1], f32, name="vs")
                nc.vector.reduce_sum(vs[:], x_t[:], axis=X_AX)
                sd = lnp.tile([P, 1], f32, name="sd")
                nc.scalar.activation(sd[:], vs[:], AF.Sqrt, bias=eps_t[:],
                                     scale=1.0 / D)
                rs = lnp.tile([P, 1], f32, name="rs")
                nc.vector.reciprocal(rs[:], sd[:])
                nc.vector.tensor_scalar_mul(xc[:], xc[:], rs[:])
                for half in range(2):
                    ps_tr = pp_ln.tile([P, 4, P], f32, name="ps_tr")
                    for j in range(4):
                        kc = half * 4 + j
                        nc.tensor.transpose(ps_tr[:, j, :],
                                            xc[:, kc * P:(kc + 1) * P], ident[:])
                    dst = xnT[:, half * 4:(half + 1) * 4, t * P:(t + 1) * P]
                    if half == 0:
                        nc.vector.tensor_copy(dst, ps_tr[:])
                    else:
                        nc.scalar.copy(dst, ps_tr[:])

            # z^T then q/k so attention can start early
            for g in range(G):
                ps_z = pp_z.tile([P, GW], f32, name="ps_z")
                for kc in range(KC):
                    nc.tensor.matmul(ps_z[:], wz_t[:, kc, :],
                                     xnT[:, kc, g * GW:(g + 1) * GW],
                                     start=(kc == 0), stop=(kc == KC - 1))
                zt_g = ztp.tile([P, GW], f32, name="zt_g")
                nc.scalar.activation(zt_g[:], ps_z[:], AF.Silu, bias=bz_sb[:])
                nc.vector.tensor_scalar(qT[:, g * GW:(g + 1) * GW], zt_g[:],
                                        gq_sb[:], bq_sb[:],
                                        op0=mybir.AluOpType.mult,
                                        op1=mybir.AluOpType.add)
                nc.vector.tensor_scalar(kT[:, g * GW:(g + 1) * GW], zt_g[:],
                                        gk_sb[:], bk_sb[:],
                                        op0=mybir.AluOpType.mult,
                                        op1=mybir.AluOpType.add)

            # u^T -> DRAM spill
            for ec in range(ECN):
                wu_t = wup.tile([P, KC, P], f32r, name="wu_t")
                nc.sync.dma_start(
                    wu_t[:],
                    wu.rearrange("(kc p) e -> p kc e", p=P)[:, :, ec * P:(ec + 1) * P])
                for g in range(G):
                    ps_u = pp_u.tile([P, GW], f32, name="ps_u")
                    for kc in range(KC):
                        nc.tensor.matmul(ps_u[:], wu_t[:, kc, :],
                                         xnT[:, kc, g * GW:(g + 1) * GW],
                                         start=(kc == 0), stop=(kc == KC - 1))
                    ut_o = utp.tile([P, GW], f32r, name="ut_o")
                    nc.scalar.activation(ut_o[:], ps_u[:], AF.Silu,
                                         bias=bu_sb[:, ec:ec + 1])
                    nc.sync.dma_start(
                        uTd[ec * P:(ec + 1) * P, g * GW:(g + 1) * GW], ut_o[:])

            # v token-major, resident
            for es in range(2):
                wv_t = wvp.tile([P, KC, EH // 2], f32r, name="wv_t")
                nc.sync.dma_start(
                    wv_t[:],
                    wv.rearrange("(kc p) e -> p kc e", p=P)[:, :, es * 512:(es + 1) * 512])
                for lc in range(LCN):
                    ps_v = pp_v.tile([P, 512], f32, name="ps_v")
                    for kc in range(KC):
                        nc.tensor.matmul(ps_v[:], xnT[:, kc, lc * P:(lc + 1) * P],
                                         wv_t[:, kc, :],
                                         start=(kc == 0),
                                         stop=(kc == KC - 1 and not has_bv))
                    if has_bv:
                        nc.tensor.matmul(ps_v[:], ones_t[:],
                                         bv_sb[:, es * 512:(es + 1) * 512],
                                         start=False, stop=True)
                    nc.scalar.activation(v_sb[:, lc, es * 512:(es + 1) * 512],
                                         ps_v[:], AF.Silu, bias=zero_t[:])

        # ---------------- phase 3: attention + output --------------------
        with ExitStack() as ctx3:
            a2p = ctx3.enter_context(tc.tile_pool(name="a2p", bufs=1))
            rp = ctx3.enter_context(tc.tile_pool(name="rp", bufs=3))
            uip = ctx3.enter_context(tc.tile_pool(name="uip", bufs=4))
            gtp = ctx3.enter_context(tc.tile_pool(name="gtp", bufs=2))
            wop = ctx3.enter_context(tc.tile_pool(name="wop", bufs=1))
            outp = ctx3.enter_context(tc.tile_pool(name="outp", bufs=3))
            pp_sc = ctx3.enter_context(tc.tile_pool(name="pp_sc", bufs=3, space="PSUM"))
            pp_av = ctx3.enter_context(tc.tile_pool(name="pp_av", bufs=2, space="PSUM"))
            pp_o = ctx3.enter_context(tc.tile_pool(name="pp_o", bufs=2, space="PSUM"))

            wo_t = wop.tile([P, ECN, D], f32r, name="wo_t")
            nc.sync.dma_start(wo_t[:], wo.rearrange("(ec p) d -> p ec d", p=P))

            for g in range(G):
                a2 = a2p.tile([P, LCN, GW], f32r, name="a2")
                for l2c in range(LCN):
                    ps_s = pp_sc.tile([P, GW], f32, name="ps_s")
                    nc.tensor.matmul(ps_s[:], kT[:, l2c * P:(l2c + 1) * P],
                                     qT[:, g * GW:(g + 1) * GW],
                                     start=True, stop=True)
                    r_t = rp.tile([P, GW], f32, name="r_t")
                    nc.scalar.activation(r_t[:], ps_s[:], AF.Relu,
                                         bias=zero_t[:], scale=1.0 / L)
                    nc.vector.tensor_tensor(a2[:, l2c, :], ps_s[:], r_t[:],
                                            mybir.AluOpType.mult)
                gt = gtp.tile([P, ECN, GW], f32r, name="gt")
                for ec in range(ECN):
                    ps_av = pp_av.tile([P, GW], f32, name="ps_av")
                    for l2c in range(LCN):
                        nc.tensor.matmul(ps_av[:], v_sb[:, l2c, ec * P:(ec + 1) * P],
                                         a2[:, l2c, :],
                                         start=(l2c == 0), stop=(l2c == LCN - 1))
                    ut_i = uip.tile([P, GW], f32r, name="ut_i")
                    nc.sync.dma_start(ut_i[:],
                                      uTd[ec * P:(ec + 1) * P, g * GW:(g + 1) * GW])
                    nc.vector.tensor_tensor(gt[:, ec, :], ps_av[:], ut_i[:],
                                            mybir.AluOpType.mult)
                for l1s in range(4):
                    for dsb in range(2):
                        ps_o = pp_o.tile([P, 512], f32, name="ps_o")
                        for ec in range(ECN):
                            nc.tensor.matmul(
                                ps_o[:], gt[:, ec, l1s * P:(l1s + 1) * P],
                                wo_t[:, ec, dsb * 512:(dsb + 1) * 512],
                                start=(ec == 0), stop=(ec == ECN - 1))
                        o_t = outp.tile([P, 512], f32, name="o_t")
                        if (l1s + dsb) % 2 == 0:
                            nc.vector.tensor_copy(o_t[:], ps_o[:])
                        else:
                            nc.scalar.copy(o_t[:], ps_o[:])
                        nc.sync.dma_start(
                            part[g * GW + l1s * P: g * GW + (l1s + 1) * P,
                                 dsb * 512:(dsb + 1) * 512], o_t[:])

    nc.finalize()
    return nc


def kernel(**inputs):
    global LAST_EXEC_NS, LAST_WALL_S
    x = np.asarray(inputs["x"], dtype=np.float32)
    ln_g = np.asarray(inputs["ln_g"], dtype=np.float32)
    ln_b = np.asarray(inputs["ln_b"], dtype=np.float32)
    W_in = np.asarray(inputs["W_in"], dtype=np.float32)
    b_in = np.asarray(inputs["b_in"], dtype=np.float32)
    W_out = np.asarray(inputs["W_out"], dtype=np.float32)
    b_out = np.asarray(inputs["b_out"], dtype=np.float32)
    gq = np.asarray(inputs["gamma_q"], dtype=np.float32)
    bq = np.asarray(inputs["beta_q"], dtype=np.float32)
    gk = np.asarray(inputs["gamma_k"], dtype=np.float32)
    bk = np.asarray(inputs["beta_k"], dtype=np.float32)

    W = W_in * ln_g[:, None]
    b_eff = ln_b @ W_in + b_in
    Wu, Wv, Wz = W[:, :E], W[:, E:2 * E], W[:, 2 * E:]
    bu_f, bv_f, bz_f = b_eff[:E], b_eff[E:2 * E], b_eff[2 * E:]
    woL = W_out * (1.0 / L)
    has_bv = bool(np.any(bv_f != 0.0))

    key = has_bv
    if key not in _CACHE:
        _CACHE[key] = _build(has_bv)
    nc = _CACHE[key]

    cc = np.ascontiguousarray
    in_maps = []
    for c in range(8):
        b, h = divmod(c, 2)
        m = {
            "x": cc(x[b]),
            "wu": cc(Wu[:, h * EH:(h + 1) * EH]),
            "wv": cc(Wv[:, h * EH:(h + 1) * EH]),
            "wz": cc(Wz),
            "wo": cc(woL[h * EH:(h + 1) * EH, :]),
            "bu": cc(bu_f[h * EH:(h + 1) * EH]),
            "bz": cc(bz_f),
            "gq": cc(gq), "bq": cc(bq), "gk": cc(gk), "bk": cc(bk),
        }
        if has_bv:
            m["bv"] = cc(bv_f[h * EH:(h + 1) * EH])
        in_maps.append(m)

    import time as _time
    global LAST_WALL_S
    _t0 = _time.time()
    res = run_bass_kernel_spmd(nc, in_maps, list(range(8)))
    LAST_WALL_S = _time.time() - _t0
    LAST_EXEC_NS = res.exec_time_ns
    out = x + b_out[None, None, :]
    for b in range(4):
        out[b] += res.results[2 * b]["part"] + res.results[2 * b + 1]["part"]
    return out.astype(np.float32)

